# revision 7
# baseline (speedup 1.0000x reference)
"""Trainium2 Bass kernel for nn_Dense_RBS_state_vector.

The RBS gate sequence collapses to a single per-basis-state diagonal scale:
    total[d] = prod_g (cos(angle_g) if mask[g,d] else 1)
    out[b,d] = x[b,d] * total[d]

Sharding: data-parallel over batch across 8 NeuronCores (1024 rows each).
The tiny [8128] scale row is computed on host (127*8128 flops of input
prep, mirroring the reference's f32 arithmetic) and replicated to every
core; each core broadcasts it across its 128 SBUF partitions with a
ones-matmul and streams its batch shard through a DVE multiply.
"""

import numpy as np

import concourse.bass as bass
import concourse.mybir as mybir
from concourse import bacc
from concourse.tile import TileContext
from concourse.bass_utils import run_bass_kernel_spmd

# Problem constants (hardcoded per harness contract; kernel.py is
# self-contained and must not read spec/reference files).
BATCH = 8192
DIM = 8128
N_GATES = 127
N_CORES = 8
ROWS_PER_CORE = BATCH // N_CORES          # 1024
P = 128                                   # SBUF partitions
ROW_TILES = ROWS_PER_CORE // P            # 8
PSUM_N = 512                              # max matmul moving free dim

_FP32 = mybir.dt.float32


def _build_program(repeat: int = 1) -> bass.Bass:
    # Bacc (not raw Bass): its compile() legalizes semaphore waits for TRN2
    # (max 1 wait per instruction), which Tile-scheduled programs need.
    nc = bacc.Bacc()
    x = nc.dram_tensor("x", [ROWS_PER_CORE, DIM], _FP32, kind="ExternalInput")
    t = nc.dram_tensor("t", [P, DIM], _FP32, kind="ExternalInput")
    out = nc.dram_tensor("out", [ROWS_PER_CORE, DIM], _FP32, kind="ExternalOutput")

    with TileContext(nc) as tc:
        with (
            tc.tile_pool(name="const", bufs=1) as const_pool,
            tc.tile_pool(name="xtiles", bufs=3) as xpool,
        ):
            # Scale row arrives pre-broadcast across partitions (4 MB, read once).
            tb = const_pool.tile([P, DIM], _FP32)
            nc.sync.dma_start(out=tb[:], in_=t[:, :])

            # Stream the batch shard: load -> scale -> store. repeat>1 is a
            # timing-only mode: the same streaming work r times in one NEFF
            # so marginal wall time per repeat isolates HW execution time.
            for _ in range(repeat):
                for i in range(ROW_TILES):
                    xt = xpool.tile([P, DIM], _FP32)
                    nc.sync.dma_start(out=xt[:], in_=x[i * P:(i + 1) * P, :])
                    nc.vector.tensor_mul(xt[:], xt[:], tb[:])
                    # Stores ride the ACT HWDGE ring so they don't queue
                    # behind the next tile's load on the SP ring.
                    nc.scalar.dma_start(out=out[i * P:(i + 1) * P, :], in_=xt[:])

    nc.finalize()
    return nc


_NC_CACHE = None


def _get_program() -> bass.Bass:
    global _NC_CACHE
    if _NC_CACHE is None:
        _NC_CACHE = _build_program()
    return _NC_CACHE


def _host_total(angles: np.ndarray, gate_masks: np.ndarray) -> np.ndarray:
    # Same f32 arithmetic as the reference.
    m = gate_masks.astype(np.float32)                        # [G, D]
    cos = np.cos(angles.astype(np.float32))                  # [G]
    scales = cos[:, None] * m + (np.float32(1.0) - m)        # [G, D]
    return np.prod(scales, axis=0, dtype=np.float32)         # [D]


def run_spmd(input_state, angles, gate_masks, **run_kwargs):
    """Shard, run on 8 cores, gather. Returns (output, BassKernelResults)."""
    x = np.ascontiguousarray(np.asarray(input_state, dtype=np.float32))
    assert x.shape == (BATCH, DIM), x.shape
    total = _host_total(np.asarray(angles), np.asarray(gate_masks))
    t_bcast = np.ascontiguousarray(np.broadcast_to(total.reshape(1, DIM), (P, DIM)))

    in_maps = [
        {
            "x": np.ascontiguousarray(x[i * ROWS_PER_CORE:(i + 1) * ROWS_PER_CORE]),
            "t": t_bcast,
        }
        for i in range(N_CORES)
    ]
    nc = _get_program()
    res = run_bass_kernel_spmd(nc, in_maps, list(range(N_CORES)), **run_kwargs)
    out = np.concatenate([np.asarray(r["out"]) for r in res.results], axis=0)
    return out, res


def kernel(input_state, angles, gate_masks):
    out, _ = run_spmd(input_state, angles, gate_masks)
    return out


# revision 10
# speedup vs baseline: 1.3752x; 1.3752x over previous
"""Trainium2 Bass kernel for nn_Dense_RBS_state_vector.

The RBS gate sequence collapses to a single per-basis-state diagonal scale:
    total[d] = prod_g (cos(angle_g) if mask[g,d] else 1)
    out[b,d] = x[b,d] * total[d]

Sharding: data-parallel over batch across 8 NeuronCores (1024 rows each).
The tiny [8128] scale row is computed on host (127*8128 flops of input
prep, mirroring the reference's f32 arithmetic) and replicated to every
core. On-core, the row is broadcast across the 128 SBUF partitions with a
ones-matmul (32 KB HBM read instead of a 4 MB pre-broadcast input), then
the batch shard streams through a DVE multiply.

Measured on the 8-core axon TRN2 slice: ~197 us steady-state per full
pass per core (66.6 MB of HBM traffic -> ~339 GB/s/core, ~95% of the
358 GB/s per-core HBM limit). Loads ride the SP HWDGE ring, stores the
ACT ring; 8.3 MB DMAs (two 128-row blocks per tile) gave the best
bidirectional bandwidth of the variants tried.
"""

import numpy as np

import concourse.bass as bass
import concourse.mybir as mybir
from concourse import bacc
from concourse.tile import TileContext
from concourse.bass_utils import run_bass_kernel_spmd

# Problem constants (hardcoded per harness contract; kernel.py is
# self-contained and must not read spec/reference files).
BATCH = 8192
DIM = 8128
N_GATES = 127
N_CORES = 8
ROWS_PER_CORE = BATCH // N_CORES          # 1024
P = 128                                   # SBUF partitions
ROW_TILES = ROWS_PER_CORE // P            # 8
BLOCKS_PER_TILE = 2                       # 128-row blocks per SBUF tile
PSUM_N = 512                              # max matmul moving free dim

_FP32 = mybir.dt.float32


def _build_program(loop_n: int | None = None) -> bass.Bass:
    # loop_n: timing-only mode - wrap the streaming stage in a device-side
    # For_i loop so one NEFF execution runs it loop_n times; the marginal
    # wall time per pass isolates steady-state HW behavior from tunnel RTT.
    # Bacc (not raw Bass): its compile() legalizes semaphore waits for TRN2
    # (max 1 wait per instruction), which Tile-scheduled programs need.
    nc = bacc.Bacc()
    x = nc.dram_tensor("x", [ROWS_PER_CORE, DIM], _FP32, kind="ExternalInput")
    t = nc.dram_tensor("t", [1, DIM], _FP32, kind="ExternalInput")
    out = nc.dram_tensor("out", [ROWS_PER_CORE, DIM], _FP32, kind="ExternalOutput")

    n_chunks = (DIM + PSUM_N - 1) // PSUM_N
    n_tiles = ROW_TILES // BLOCKS_PER_TILE

    # Row r = a*128 + p of the shard lives at tile slot [p, a].
    xr = x.rearrange("(a p) d -> p a d", p=P)
    outr = out.rearrange("(a p) d -> p a d", p=P)

    with TileContext(nc) as tc:
        with (
            tc.tile_pool(name="const", bufs=1) as const_pool,
            tc.tile_pool(name="xtiles", bufs=2) as xpool,
            tc.tile_pool(name="psum", bufs=4, space="PSUM") as psum_pool,
        ):
            ones = const_pool.tile([1, P], _FP32)
            nc.vector.memset(ones[:], 1.0)

            # The scale row lands in tb's row 0, then ones[1,128].T @ row
            # broadcasts it across all 128 partitions chunk by chunk
            # (PSUM bank = 512 f32). The copy overwrites row 0 with its
            # own value after the matmul read - Tile serializes that WAR.
            tb = const_pool.tile([P, DIM], _FP32)
            nc.sync.dma_start(out=tb[0:1, :], in_=t[:, :])
            for c in range(n_chunks):
                lo = c * PSUM_N
                hi = min(lo + PSUM_N, DIM)
                ps = psum_pool.tile([P, hi - lo], _FP32)
                nc.tensor.matmul(ps[:], ones[:], tb[0:1, lo:hi],
                                 start=True, stop=True)
                nc.vector.tensor_copy(tb[:, lo:hi], ps[:])

            # Stream the batch shard: load -> scale -> store, two 128-row
            # blocks per 8.3 MB DMA. Stores ride the ACT HWDGE ring so
            # they don't queue behind the next tile's load on the SP ring.
            def stream_pass():
                for i in range(n_tiles):
                    a0 = i * BLOCKS_PER_TILE
                    a1 = a0 + BLOCKS_PER_TILE
                    xt = xpool.tile([P, BLOCKS_PER_TILE, DIM], _FP32)
                    nc.sync.dma_start(out=xt[:], in_=xr[:, a0:a1, :])
                    for a in range(BLOCKS_PER_TILE):
                        nc.vector.tensor_mul(xt[:, a, :], xt[:, a, :], tb[:])
                    nc.scalar.dma_start(out=outr[:, a0:a1, :], in_=xt[:])

            if loop_n is None:
                stream_pass()
            else:
                with tc.For_i(0, loop_n, 1):
                    stream_pass()

    nc.finalize()
    return nc


_NC_CACHE = None


def _get_program() -> bass.Bass:
    global _NC_CACHE
    if _NC_CACHE is None:
        _NC_CACHE = _build_program()
    return _NC_CACHE


def _host_total(angles: np.ndarray, gate_masks: np.ndarray) -> np.ndarray:
    # Same f32 arithmetic as the reference.
    m = gate_masks.astype(np.float32)                        # [G, D]
    cos = np.cos(angles.astype(np.float32))                  # [G]
    scales = cos[:, None] * m + (np.float32(1.0) - m)        # [G, D]
    return np.prod(scales, axis=0, dtype=np.float32)         # [D]


def make_in_maps(input_state, angles, gate_masks):
    x = np.ascontiguousarray(np.asarray(input_state, dtype=np.float32))
    assert x.shape == (BATCH, DIM), x.shape
    total = _host_total(np.asarray(angles), np.asarray(gate_masks))
    trow = np.ascontiguousarray(total.reshape(1, DIM))
    return [
        {
            "x": np.ascontiguousarray(x[i * ROWS_PER_CORE:(i + 1) * ROWS_PER_CORE]),
            "t": trow,
        }
        for i in range(N_CORES)
    ]


def run_spmd(input_state, angles, gate_masks, **run_kwargs):
    """Shard, run on 8 cores, gather. Returns (output, BassKernelResults)."""
    in_maps = make_in_maps(input_state, angles, gate_masks)
    nc = _get_program()
    res = run_bass_kernel_spmd(nc, in_maps, list(range(N_CORES)), **run_kwargs)
    out = np.concatenate([np.asarray(r["out"]) for r in res.results], axis=0)
    return out, res


def kernel(input_state, angles, gate_masks):
    out, _ = run_spmd(input_state, angles, gate_masks)
    return out
